# revision 20
# baseline (speedup 1.0000x reference)
"""Trainium2 Bass kernel for nn_Node2Pair_bias (LayerNorm -> dual projection ->
pair outer-product -> head-mix linear).

Reference computation (B=2, L=512, D=256, DH=32, H=16, K=2, P=128):
    x   = LayerNorm(node) * gamma + beta, masked        [B, L, D]
    left  = (x @ W_left + b_left)                       [B, L, DH] -> [B,L,H,K]
    right = (x @ W_right + b_right)/sqrt(DH)            [B, L, DH] -> [B,L,H,K]
    out[b,i,j,h] = sum_k left[b,i,h,k]*right[b,j,h,k]
    out[b,i,j,p] = sum_h out[b,i,j,h]*W_out[h,p] + b_out[p]   [B, L, L, P]

Mathematical restructuring used here (c = (h,k) combined channel, 0..31):
    out[b,i,j,p] = sum_c right[b,j,c] * (left[b,i,c] * W2[c,p]) + b_out[p]
with W2[c,p] = W_out[c//2, p].  For each i we form M_i[c,p] = left[b,i,c]*W2[c,p]
on the vector engine, pack 4 i's side by side into an rhs of [33, 512]
(row 32 = b_out, matched by a constant-1 row 32 appended to rightT), and do one
fp32r matmul  lhsT=rightT[33, j-chunk 128]  x  rhs=M_pack[33, 512]
-> psum[j=128, (i4, p)=512].  PSUM is copied to SBUF staging (16 i's = 1 MiB)
and DMA'd to the output shard.

Sharding: the i axis of L is split across the 8 cores (sequence-parallel); each
core holds its [B, 64] slice of `left` inputs plus the full `right` side and
writes a [B, 64, L, P] output shard.  No cross-device communication.

LayerNorm gamma/beta are folded into the projection weights on the host
(exact algebra): W_e = gamma[:,None]*W, with an extra K=1 accumulation row
carrying beta@W * mask (mask enters as a 0/1 row since masking commutes with
the diagonal gamma scaling).
"""

import os
import sys

sys.path.insert(0, "/opt/trn_rl_repo")

import numpy as np

import concourse.bass as bass
import concourse.mybir as mybir
import concourse.tile as tile
from concourse import bacc
from concourse.bass_utils import run_bass_kernel_spmd
from concourse.masks import make_identity

F32 = mybir.dt.float32
F32R = mybir.dt.float32r
BF16 = mybir.dt.bfloat16

B, L, D = 2, 512, 256
DH, H, PAIR = 32, 16, 128
NCORES = 8
LSH = L // NCORES          # 64 i's per core per batch
LN_EPS = 1e-5

_COMPILED = None  # (nc, input_names)


def _build_program():
    nc = bacc.Bacc("TRN2", target_bir_lowering=False, debug=False,
                   num_devices=NCORES)

    # ---------------- DRAM parameters ----------------
    def din(name, shape):
        return nc.dram_tensor(name, list(shape), F32, kind="ExternalInput").ap()

    node_full = din("node_full", (B * L, D))        # all rows, (b,l) major
    node_shard = din("node_shard", (B * LSH, D))    # this core's i rows, (b,i)
    mask_col_full = din("mask_col_full", (128, B * L // 128))  # [:, t] = tile t
    mask_col_shard = din("mask_col_shard", (128, 1))
    mask_row_full = din("mask_row_full", (B, L))    # 0/1 rows per batch
    mask_row_shard = din("mask_row_shard", (1, B * LSH))
    w_left_e = din("w_left_e", (D + 1, DH))         # rows: gamma*W_l; last: beta@W_l
    w_right_e = din("w_right_e", (D + 1, DH))       # scaled by 1/sqrt(DH)
    b_left_col = din("b_left_col", (DH, 1))
    b_right_col = din("b_right_col", (DH, 1))       # scaled by 1/sqrt(DH)
    w2 = din("w2", (DH + 1, PAIR))      # W_out rows repeated x2, then b_out row

    # Permuted output layout: [b, jc, sg, j, i16, p] — every staging buffer
    # lands as one fully contiguous 1 MiB stream (8 KiB per partition run).
    # The host un-permutes while assembling the full output.
    out = nc.dram_tensor("out", [B, 4, 4, 128, 16, PAIR], F32,
                         kind="ExternalOutput").ap()

    NT_FULL = B * L // 128   # 8 LayerNorm tiles for the full sequence

    with tile.TileContext(nc) as tc:
        with (
            tc.tile_pool(name="singles", bufs=1) as singles,
            tc.tile_pool(name="xpool", bufs=9) as xpool,
            tc.tile_pool(name="stats", bufs=4) as stats,
            tc.tile_pool(name="xt", bufs=1) as xt_pool,
            tc.tile_pool(name="persist", bufs=1) as persist,
            tc.tile_pool(name="mp", bufs=8) as mp_pool,
            tc.tile_pool(name="stag", bufs=8) as stag_pool,
            tc.tile_pool(name="ps_tp", bufs=1, space="PSUM") as ps_tp,
            tc.tile_pool(name="ps_proj", bufs=1, space="PSUM") as ps_proj,
            tc.tile_pool(name="ps_big", bufs=5, space="PSUM") as ps_big,
        ):
            # ---------------- constants ----------------
            ident = singles.tile([128, 128], F32, tag="ident")
            make_identity(nc, ident)
            eps_t = singles.tile([128, 1], F32, tag="eps")
            nc.vector.memset(eps_t, LN_EPS)

            # hot-path loads on sync (HWDGE) in dependency-critical order;
            # right-side constants via gpsimd (SWDGE) in parallel
            xs = xpool.tile([128, D], F32, tag="x", name="xs")
            nc.sync.dma_start(out=xs, in_=node_shard[:, :])
            mcs_sb = singles.tile([128, 1], F32, tag="mcs")
            nc.sync.dma_start(out=mcs_sb, in_=mask_col_shard[:, :])
            xf_tiles = [None] * NT_FULL
            for t in range(4):
                xf = xpool.tile([128, D], F32, tag="x", name=f"xf{t}")
                nc.sync.dma_start(out=xf,
                                  in_=node_full[t * 128:(t + 1) * 128, :])
                xf_tiles[t] = xf
            mcf_sb = singles.tile([128, NT_FULL], F32, tag="mcf")
            nc.sync.dma_start(out=mcf_sb, in_=mask_col_full[:, :])
            wl_sb = [singles.tile([128, DH], F32, tag=f"wl{dc}", name=f"wl{dc}")
                     for dc in range(2)]
            wl_row = singles.tile([1, DH], F32, tag="wlrow")
            for dc in range(2):
                nc.sync.dma_start(out=wl_sb[dc],
                                  in_=w_left_e[dc * 128:(dc + 1) * 128, :])
            nc.sync.dma_start(out=wl_row, in_=w_left_e[D:D + 1, :])
            bl_sb = singles.tile([DH, 1], F32, tag="bl")
            nc.sync.dma_start(out=bl_sb, in_=b_left_col[:, :])
            w2_sb = singles.tile([DH + 1, PAIR], F32, tag="w2")
            nc.sync.dma_start(out=w2_sb, in_=w2[:, :])
            for t in range(4, NT_FULL):
                xf = xpool.tile([128, D], F32, tag="x", name=f"xf{t}")
                nc.sync.dma_start(out=xf,
                                  in_=node_full[t * 128:(t + 1) * 128, :])
                xf_tiles[t] = xf

            wr_sb = [singles.tile([128, DH], F32, tag=f"wr{dc}", name=f"wr{dc}")
                     for dc in range(2)]
            wr_row = singles.tile([1, DH], F32, tag="wrrow")
            for dc in range(2):
                nc.gpsimd.dma_start(out=wr_sb[dc],
                                    in_=w_right_e[dc * 128:(dc + 1) * 128, :])
            nc.gpsimd.dma_start(out=wr_row, in_=w_right_e[D:D + 1, :])
            br_sb = singles.tile([DH, 1], F32, tag="br")
            nc.gpsimd.dma_start(out=br_sb, in_=b_right_col[:, :])
            mrf_sb = [singles.tile([1, L], F32, tag=f"mrf{b}", name=f"mrf{b}")
                      for b in range(B)]
            for b in range(B):
                nc.gpsimd.dma_start(out=mrf_sb[b],
                                    in_=mask_row_full[b:b + 1, :])
            mrs_sb = singles.tile([1, B * LSH], F32, tag="mrs")
            nc.gpsimd.dma_start(out=mrs_sb, in_=mask_row_shard[:, :])
            ones_row = singles.tile([1, L], F32, tag="ones")
            nc.vector.memset(ones_row, 1.0)

            warm_w = singles.tile([128, 128], BF16, tag="warmw")
            nc.vector.memset(warm_w, 1.0)
            warm_rhs = singles.tile([128, 512], BF16, tag="warmr")
            nc.vector.memset(warm_rhs, 1.0)
            warm_ps = ps_big.tile([128, 512], F32, tag="big", name="warm_ps")
            for _ in range(20):
                nc.tensor.matmul(warm_ps, warm_w, warm_rhs,
                                 start=True, stop=True)

            # ---------------- LayerNorm helper ----------------
            def layernorm_masked(x_t, mask_col_ap):
                """x_t [128, D] in place -> (x - mu) * rsqrt(var+eps) * mask."""
                st = stats.tile([128, 6], F32, tag="st")
                nc.vector.bn_stats(out=st, in_=x_t)
                mv = stats.tile([128, 2], F32, tag="mv")
                nc.vector.bn_aggr(out=mv, in_=st)
                sd = stats.tile([128, 1], F32, tag="sd")
                nc.scalar.activation(out=sd, in_=mv[:, 1:2],
                                     func=mybir.ActivationFunctionType.Sqrt,
                                     bias=eps_t, scale=1.0)
                rs = stats.tile([128, 1], F32, tag="rs")
                nc.vector.reciprocal(out=rs, in_=sd)
                rsm = stats.tile([128, 1], F32, tag="rsm")
                nc.vector.tensor_mul(out=rsm, in0=rs, in1=mask_col_ap)
                nc.vector.tensor_scalar(out=x_t, in0=x_t,
                                        scalar1=mv[:, 0:1], scalar2=rsm,
                                        op0=mybir.AluOpType.subtract,
                                        op1=mybir.AluOpType.mult)

            # ---------------- shard path: leftT_all [DH, B*LSH] ----------------
            layernorm_masked(xs, mcs_sb[:, 0:1])

            xsT = [persist.tile([128, B * LSH], F32, tag=f"xsT{dc}", name=f"xsT{dc}")
                   for dc in range(2)]
            for dc in range(2):
                pt = ps_tp.tile([128, 128], F32, tag="tp")
                nc.tensor.transpose(pt, xs[:, dc * 128:(dc + 1) * 128], ident)
                nc.scalar.copy(out=xsT[dc], in_=pt)

            ps_l = ps_proj.tile([DH, L], F32, tag="pr", name="ps_l")
            ps_l = ps_l[:, 0:B * LSH]
            for dc in range(2):
                nc.tensor.matmul(ps_l, wl_sb[dc], xsT[dc],
                                 start=(dc == 0), stop=False)
            nc.tensor.matmul(ps_l, wl_row, mrs_sb, start=False, stop=True)
            leftT = persist.tile([DH + 1, B * LSH], F32, tag="leftT")
            nc.vector.tensor_scalar_add(out=leftT[0:DH, :], in0=ps_l,
                                        scalar1=bl_sb)
            nc.vector.memset(leftT[DH:DH + 1, :], 1.0)

            # ---------------- full path: rightT_r[b] [33, L] fp32r ----------------
            rightT = [persist.tile([DH + 1, L], BF16, tag=f"rt{b}", name=f"rt{b}")
                      for b in range(B)]
            xT = [[persist.tile([128, L], F32, tag=f"xT{b}_{dc}", name=f"xT{b}_{dc}")
                   for dc in range(2)] for b in range(B)]
            for b in range(B):
                for lc in range(4):
                    t = b * 4 + lc
                    xf = xf_tiles[t]
                    layernorm_masked(xf, mcf_sb[:, t:t + 1])
                    for dc in range(2):
                        pt = ps_tp.tile([128, 128], F32, tag="tp")
                        nc.tensor.transpose(pt, xf[:, dc * 128:(dc + 1) * 128],
                                            ident)
                        eng = nc.vector if (lc + dc) % 2 == 0 else nc.scalar
                        if eng is nc.vector:
                            nc.vector.tensor_copy(
                                out=xT[b][dc][:, lc * 128:(lc + 1) * 128],
                                in_=pt)
                        else:
                            nc.scalar.copy(
                                out=xT[b][dc][:, lc * 128:(lc + 1) * 128],
                                in_=pt)

                ps_r = ps_proj.tile([DH, L], F32, tag="pr")
                for jc in range(4):
                    jsl = slice(jc * 128, (jc + 1) * 128)
                    for dc in range(2):
                        nc.tensor.matmul(ps_r[:, jsl], wr_sb[dc],
                                         xT[b][dc][:, jsl],
                                         start=(dc == 0), stop=False)
                    nc.tensor.matmul(ps_r[:, jsl], wr_row, mrf_sb[b][:, jsl],
                                     start=False, stop=True)
                    nc.vector.tensor_scalar_add(out=rightT[b][0:DH, jsl],
                                                in0=ps_r[:, jsl],
                                                scalar1=br_sb)
                nc.vector.tensor_copy(out=rightT[b][DH:DH + 1, :],
                                      in_=ones_row)

            # ---------------- main pair loop ----------------
            # sg-outer: per sg build the 4 M_packs it needs (split between
            # DVE and ACT so neither bursts), then 4 j-chunks x 4 matmuls,
            # each j-chunk staging 1 MiB and streaming it out.
            def build_mps(b, sg):
                mps = []
                for il in range(4):
                    mp = mp_pool.tile([DH + 1, 512], BF16, tag="mp",
                                      name=f"mp{b}_{sg}_{il}")
                    for q in range(4):
                        col = b * LSH + (sg * 4 + il) * 4 + q
                        osl = mp[:, q * 128:(q + 1) * 128]
                        sc = leftT[:, col:col + 1]
                        if q == 1 or q == 3:
                            nc.scalar.activation(
                                out=osl, in_=w2_sb,
                                func=mybir.ActivationFunctionType.Copy,
                                scale=sc)
                        else:
                            nc.vector.tensor_scalar_mul(
                                out=osl, in0=w2_sb, scalar1=sc)
                    mps.append(mp)
                return mps

            for b in range(B):
                for sg in range(4):
                    mps = build_mps(b, sg)
                    for jc in range(4):
                        lhsT = rightT[b][:, jc * 128:(jc + 1) * 128]
                        stg = stag_pool.tile([128, 16 * 128], F32, tag="stag")
                        for il in range(4):
                            pb = ps_big.tile([128, 512], F32, tag="big")
                            nc.tensor.matmul(pb, lhsT, mps[il],
                                             start=True, stop=True)
                            dst = stg[:, il * 512:(il + 1) * 512]
                            if il % 2 == 0:
                                nc.vector.tensor_copy(out=dst, in_=pb)
                            else:
                                nc.scalar.copy(out=dst, in_=pb)
                        dst_ap = out[b, jc, sg, :, :, :]
                        src_ap = stg[:, :].rearrange("j (i p) -> j i p", p=128)
                        deng = nc.sync if jc % 2 == 0 else nc.gpsimd
                        deng.dma_start(out=dst_ap, in_=src_ap)

    nc.compile()
    names = ["node_full", "node_shard", "mask_col_full", "mask_col_shard",
             "mask_row_full", "mask_row_shard", "w_left_e", "w_right_e",
             "b_left_col", "b_right_col", "w2"]
    return nc, names


def _prepare_in_maps(node, mask, ln_gamma, ln_beta, W_left, b_left, W_right,
                     b_right, W_out, b_out):
    f = np.float32
    node = np.ascontiguousarray(np.asarray(node, dtype=f))        # [B, L, D]
    mask_f = np.asarray(mask).astype(f)                           # [B, L]
    gamma = np.asarray(ln_gamma, dtype=f)
    beta = np.asarray(ln_beta, dtype=f)
    W_l = np.asarray(W_left, dtype=f)
    W_r = np.asarray(W_right, dtype=f)
    b_l = np.asarray(b_left, dtype=f)
    b_r = np.asarray(b_right, dtype=f)
    W_o = np.asarray(W_out, dtype=f)
    b_o = np.asarray(b_out, dtype=f)

    s = 1.0 / np.sqrt(np.float32(DH))
    w_left_e = np.concatenate([gamma[:, None] * W_l, (beta @ W_l)[None, :]], 0)
    w_right_e = np.concatenate([gamma[:, None] * W_r, (beta @ W_r)[None, :]],
                               0) * s
    w2 = np.concatenate([np.repeat(W_o, 2, axis=0), b_o[None, :]], 0)

    node_flat = node.reshape(B * L, D)
    mask_col_full = np.ascontiguousarray(mask_f.reshape(-1, 128).T)  # [128, 8]
    mask_row_full = np.ascontiguousarray(mask_f)                     # [B, L]

    common = {
        "node_full": node_flat,
        "mask_col_full": mask_col_full,
        "mask_row_full": mask_row_full,
        "w_left_e": np.ascontiguousarray(w_left_e),
        "w_right_e": np.ascontiguousarray(w_right_e),
        "b_left_col": np.ascontiguousarray(b_l[:, None]),
        "b_right_col": np.ascontiguousarray(b_r[:, None] * s),
        "w2": np.ascontiguousarray(w2),
    }

    in_maps = []
    for c in range(NCORES):
        sl = slice(c * LSH, (c + 1) * LSH)
        shard = np.ascontiguousarray(node[:, sl, :].reshape(B * LSH, D))
        msk = mask_f[:, sl]                                       # [B, LSH]
        m = dict(common)
        m["node_shard"] = shard
        m["mask_col_shard"] = np.ascontiguousarray(msk.reshape(-1)[:, None])
        m["mask_row_shard"] = np.ascontiguousarray(msk.reshape(1, -1))
        in_maps.append(m)
    return in_maps


def kernel(**inputs):
    global _COMPILED
    if _COMPILED is None:
        _COMPILED = _build_program()
    nc, names = _COMPILED
    in_maps = _prepare_in_maps(**inputs)
    res = run_bass_kernel_spmd(nc, in_maps, core_ids=list(range(NCORES)))
    full = np.empty((B, L, L, PAIR), np.float32)
    for c in range(NCORES):
        dev = res.results[c]["out"]   # [b, jc, sg, j, i16, p]
        full[:, c * LSH:(c + 1) * LSH] = (
            dev.transpose(0, 2, 4, 1, 3, 5).reshape(B, LSH, L, PAIR))
    return full


if __name__ == "__main__":
    rng = np.random.default_rng(0)
    inputs = {
        "node": rng.standard_normal((B, L, D)).astype(np.float32),
        "mask": np.ones((B, L), dtype=bool),
        "ln_gamma": np.ones(D, np.float32),
        "ln_beta": np.zeros(D, np.float32),
        "W_left": (rng.standard_normal((D, DH)) / np.sqrt(D)).astype(np.float32),
        "b_left": np.zeros(DH, np.float32),
        "W_right": (rng.standard_normal((D, DH)) / np.sqrt(D)).astype(np.float32),
        "b_right": np.zeros(DH, np.float32),
        "W_out": (rng.standard_normal((H, PAIR)) / np.sqrt(H)).astype(np.float32),
        "b_out": np.zeros(PAIR, np.float32),
    }
    got = kernel(**inputs)
    print("kernel output:", got.shape, got.dtype)


# revision 21
# speedup vs baseline: 1.0721x; 1.0721x over previous
"""Trainium2 Bass kernel for nn_Node2Pair_bias (LayerNorm -> dual projection ->
pair outer-product -> head-mix linear).

Reference computation (B=2, L=512, D=256, DH=32, H=16, K=2, P=128):
    x   = LayerNorm(node) * gamma + beta, masked        [B, L, D]
    left  = (x @ W_left + b_left)                       [B, L, DH] -> [B,L,H,K]
    right = (x @ W_right + b_right)/sqrt(DH)            [B, L, DH] -> [B,L,H,K]
    out[b,i,j,h] = sum_k left[b,i,h,k]*right[b,j,h,k]
    out[b,i,j,p] = sum_h out[b,i,j,h]*W_out[h,p] + b_out[p]   [B, L, L, P]

Mathematical restructuring used here (c = (h,k) combined channel, 0..31):
    out[b,i,j,p] = sum_c right[b,j,c] * (left[b,i,c] * W2[c,p]) + b_out[p]
with W2[c,p] = W_out[c//2, p].  For each i we form M_i[c,p] = left[b,i,c]*W2[c,p]
on the vector engine, pack 4 i's side by side into an rhs of [33, 512]
(row 32 = b_out, matched by a constant-1 row 32 appended to rightT), and do one
fp32r matmul  lhsT=rightT[33, j-chunk 128]  x  rhs=M_pack[33, 512]
-> psum[j=128, (i4, p)=512].  PSUM is copied to SBUF staging (16 i's = 1 MiB)
and DMA'd to the output shard.

Sharding: the i axis of L is split across the 8 cores (sequence-parallel); each
core holds its [B, 64] slice of `left` inputs plus the full `right` side and
writes a [B, 64, L, P] output shard.  No cross-device communication.

LayerNorm gamma/beta are folded into the projection weights on the host
(exact algebra): W_e = gamma[:,None]*W, with an extra K=1 accumulation row
carrying beta@W * mask (mask enters as a 0/1 row since masking commutes with
the diagonal gamma scaling).
"""

import os
import sys

sys.path.insert(0, "/opt/trn_rl_repo")

import numpy as np

import concourse.bass as bass
import concourse.mybir as mybir
import concourse.tile as tile
from concourse import bacc
from concourse.bass_utils import run_bass_kernel_spmd
from concourse.masks import make_identity

F32 = mybir.dt.float32
F32R = mybir.dt.float32r
BF16 = mybir.dt.bfloat16

B, L, D = 2, 512, 256
DH, H, PAIR = 32, 16, 128
NCORES = 8
LSH = L // NCORES          # 64 i's per core per batch
LN_EPS = 1e-5

_COMPILED = None  # (nc, input_names)


def _build_program():
    nc = bacc.Bacc("TRN2", target_bir_lowering=False, debug=False,
                   num_devices=NCORES)

    # ---------------- DRAM parameters ----------------
    def din(name, shape):
        return nc.dram_tensor(name, list(shape), F32, kind="ExternalInput").ap()

    node_full = din("node_full", (B * L, D))        # all rows, (b,l) major
    node_shard = din("node_shard", (B * LSH, D))    # this core's i rows, (b,i)
    mask_col_full = din("mask_col_full", (128, B * L // 128))  # [:, t] = tile t
    mask_col_shard = din("mask_col_shard", (128, 1))
    mask_row_full = din("mask_row_full", (B, L))    # 0/1 rows per batch
    mask_row_shard = din("mask_row_shard", (1, B * LSH))
    w_left_e = din("w_left_e", (D + 1, DH))         # rows: gamma*W_l; last: beta@W_l
    w_right_e = din("w_right_e", (D + 1, DH))       # scaled by 1/sqrt(DH)
    b_left_col = din("b_left_col", (DH, 1))
    b_right_col = din("b_right_col", (DH, 1))       # scaled by 1/sqrt(DH)
    w2 = din("w2", (DH + 1, PAIR))      # W_out rows repeated x2, then b_out row

    # Permuted output layout: [b, jc, sg, j, i16, p] — every staging buffer
    # lands as one fully contiguous 1 MiB stream (8 KiB per partition run).
    # The host un-permutes while assembling the full output.
    out = nc.dram_tensor("out", [B, 4, 4, 128, 16, PAIR], F32,
                         kind="ExternalOutput").ap()

    NT_FULL = B * L // 128   # 8 LayerNorm tiles for the full sequence

    with tile.TileContext(nc) as tc:
        with (
            tc.tile_pool(name="singles", bufs=1) as singles,
            tc.tile_pool(name="xpool", bufs=9) as xpool,
            tc.tile_pool(name="stats", bufs=4) as stats,
            tc.tile_pool(name="xt", bufs=1) as xt_pool,
            tc.tile_pool(name="persist", bufs=1) as persist,
            tc.tile_pool(name="mp", bufs=8) as mp_pool,
            tc.tile_pool(name="stag", bufs=8) as stag_pool,
            tc.tile_pool(name="ps_tp", bufs=1, space="PSUM") as ps_tp,
            tc.tile_pool(name="ps_proj", bufs=1, space="PSUM") as ps_proj,
            tc.tile_pool(name="ps_big", bufs=5, space="PSUM") as ps_big,
        ):
            # ---------------- constants ----------------
            ident = singles.tile([128, 128], F32, tag="ident")
            make_identity(nc, ident)
            eps_t = singles.tile([128, 1], F32, tag="eps")
            nc.vector.memset(eps_t, LN_EPS)

            # hot-path loads on sync (HWDGE) in dependency-critical order;
            # right-side constants via gpsimd (SWDGE) in parallel
            xs = xpool.tile([128, D], F32, tag="x", name="xs")
            nc.sync.dma_start(out=xs, in_=node_shard[:, :])
            mcs_sb = singles.tile([128, 1], F32, tag="mcs")
            nc.sync.dma_start(out=mcs_sb, in_=mask_col_shard[:, :])
            xf_tiles = [None] * NT_FULL
            for t in range(4):
                xf = xpool.tile([128, D], F32, tag="x", name=f"xf{t}")
                nc.sync.dma_start(out=xf,
                                  in_=node_full[t * 128:(t + 1) * 128, :])
                xf_tiles[t] = xf
            mcf_sb = singles.tile([128, NT_FULL], F32, tag="mcf")
            nc.sync.dma_start(out=mcf_sb, in_=mask_col_full[:, :])
            wl_sb = [singles.tile([128, DH], F32, tag=f"wl{dc}", name=f"wl{dc}")
                     for dc in range(2)]
            wl_row = singles.tile([1, DH], F32, tag="wlrow")
            for dc in range(2):
                nc.sync.dma_start(out=wl_sb[dc],
                                  in_=w_left_e[dc * 128:(dc + 1) * 128, :])
            nc.sync.dma_start(out=wl_row, in_=w_left_e[D:D + 1, :])
            bl_sb = singles.tile([DH, 1], F32, tag="bl")
            nc.sync.dma_start(out=bl_sb, in_=b_left_col[:, :])
            w2_sb = singles.tile([DH + 1, PAIR], F32, tag="w2")
            nc.sync.dma_start(out=w2_sb, in_=w2[:, :])
            for t in range(4, NT_FULL):
                xf = xpool.tile([128, D], F32, tag="x", name=f"xf{t}")
                nc.sync.dma_start(out=xf,
                                  in_=node_full[t * 128:(t + 1) * 128, :])
                xf_tiles[t] = xf

            wr_sb = [singles.tile([128, DH], F32, tag=f"wr{dc}", name=f"wr{dc}")
                     for dc in range(2)]
            wr_row = singles.tile([1, DH], F32, tag="wrrow")
            for dc in range(2):
                nc.gpsimd.dma_start(out=wr_sb[dc],
                                    in_=w_right_e[dc * 128:(dc + 1) * 128, :])
            nc.gpsimd.dma_start(out=wr_row, in_=w_right_e[D:D + 1, :])
            br_sb = singles.tile([DH, 1], F32, tag="br")
            nc.gpsimd.dma_start(out=br_sb, in_=b_right_col[:, :])
            mrf_sb = [singles.tile([1, L], F32, tag=f"mrf{b}", name=f"mrf{b}")
                      for b in range(B)]
            for b in range(B):
                nc.gpsimd.dma_start(out=mrf_sb[b],
                                    in_=mask_row_full[b:b + 1, :])
            mrs_sb = singles.tile([1, B * LSH], F32, tag="mrs")
            nc.gpsimd.dma_start(out=mrs_sb, in_=mask_row_shard[:, :])
            ones_row = singles.tile([1, L], F32, tag="ones")
            nc.vector.memset(ones_row, 1.0)

            # ---------------- LayerNorm helper ----------------
            def layernorm_masked(x_t, mask_col_ap):
                """x_t [128, D] in place -> (x - mu) * rsqrt(var+eps) * mask."""
                st = stats.tile([128, 6], F32, tag="st")
                nc.vector.bn_stats(out=st, in_=x_t)
                mv = stats.tile([128, 2], F32, tag="mv")
                nc.vector.bn_aggr(out=mv, in_=st)
                sd = stats.tile([128, 1], F32, tag="sd")
                nc.scalar.activation(out=sd, in_=mv[:, 1:2],
                                     func=mybir.ActivationFunctionType.Sqrt,
                                     bias=eps_t, scale=1.0)
                rs = stats.tile([128, 1], F32, tag="rs")
                nc.vector.reciprocal(out=rs, in_=sd)
                rsm = stats.tile([128, 1], F32, tag="rsm")
                nc.vector.tensor_mul(out=rsm, in0=rs, in1=mask_col_ap)
                nc.vector.tensor_scalar(out=x_t, in0=x_t,
                                        scalar1=mv[:, 0:1], scalar2=rsm,
                                        op0=mybir.AluOpType.subtract,
                                        op1=mybir.AluOpType.mult)

            # ---------------- shard path: leftT_all [DH, B*LSH] ----------------
            layernorm_masked(xs, mcs_sb[:, 0:1])

            xsT = [persist.tile([128, B * LSH], F32, tag=f"xsT{dc}", name=f"xsT{dc}")
                   for dc in range(2)]
            for dc in range(2):
                pt = ps_tp.tile([128, 128], F32, tag="tp")
                nc.tensor.transpose(pt, xs[:, dc * 128:(dc + 1) * 128], ident)
                nc.scalar.copy(out=xsT[dc], in_=pt)

            ps_l = ps_proj.tile([DH, L], F32, tag="pr", name="ps_l")
            ps_l = ps_l[:, 0:B * LSH]
            for dc in range(2):
                nc.tensor.matmul(ps_l, wl_sb[dc], xsT[dc],
                                 start=(dc == 0), stop=False)
            nc.tensor.matmul(ps_l, wl_row, mrs_sb, start=False, stop=True)
            leftT = persist.tile([DH + 1, B * LSH], F32, tag="leftT")
            nc.vector.tensor_scalar_add(out=leftT[0:DH, :], in0=ps_l,
                                        scalar1=bl_sb)
            nc.vector.memset(leftT[DH:DH + 1, :], 1.0)

            # ---------------- full path: rightT_r[b] [33, L] fp32r ----------------
            rightT = [persist.tile([DH + 1, L], BF16, tag=f"rt{b}", name=f"rt{b}")
                      for b in range(B)]
            xT = [[persist.tile([128, L], F32, tag=f"xT{b}_{dc}", name=f"xT{b}_{dc}")
                   for dc in range(2)] for b in range(B)]
            for b in range(B):
                for lc in range(4):
                    t = b * 4 + lc
                    xf = xf_tiles[t]
                    layernorm_masked(xf, mcf_sb[:, t:t + 1])
                    for dc in range(2):
                        pt = ps_tp.tile([128, 128], F32, tag="tp")
                        nc.tensor.transpose(pt, xf[:, dc * 128:(dc + 1) * 128],
                                            ident)
                        eng = nc.vector if (lc + dc) % 2 == 0 else nc.scalar
                        if eng is nc.vector:
                            nc.vector.tensor_copy(
                                out=xT[b][dc][:, lc * 128:(lc + 1) * 128],
                                in_=pt)
                        else:
                            nc.scalar.copy(
                                out=xT[b][dc][:, lc * 128:(lc + 1) * 128],
                                in_=pt)

                ps_r = ps_proj.tile([DH, L], F32, tag="pr")
                for jc in range(4):
                    jsl = slice(jc * 128, (jc + 1) * 128)
                    for dc in range(2):
                        nc.tensor.matmul(ps_r[:, jsl], wr_sb[dc],
                                         xT[b][dc][:, jsl],
                                         start=(dc == 0), stop=False)
                    nc.tensor.matmul(ps_r[:, jsl], wr_row, mrf_sb[b][:, jsl],
                                     start=False, stop=True)
                    nc.vector.tensor_scalar_add(out=rightT[b][0:DH, jsl],
                                                in0=ps_r[:, jsl],
                                                scalar1=br_sb)
                nc.vector.tensor_copy(out=rightT[b][DH:DH + 1, :],
                                      in_=ones_row)

            # ---------------- main pair loop ----------------
            # sg-outer: per sg build the 4 M_packs it needs (split between
            # DVE and ACT so neither bursts), then 4 j-chunks x 4 matmuls,
            # each j-chunk staging 1 MiB and streaming it out.
            def build_mps(b, sg):
                mps = []
                for il in range(4):
                    mp = mp_pool.tile([DH + 1, 512], BF16, tag="mp",
                                      name=f"mp{b}_{sg}_{il}")
                    for q in range(4):
                        col = b * LSH + (sg * 4 + il) * 4 + q
                        osl = mp[:, q * 128:(q + 1) * 128]
                        sc = leftT[:, col:col + 1]
                        if q == 1 or q == 3:
                            nc.scalar.activation(
                                out=osl, in_=w2_sb,
                                func=mybir.ActivationFunctionType.Copy,
                                scale=sc)
                        else:
                            nc.vector.tensor_scalar_mul(
                                out=osl, in0=w2_sb, scalar1=sc)
                    mps.append(mp)
                return mps

            for b in range(B):
                for sg in range(4):
                    mps = build_mps(b, sg)
                    for jc in range(4):
                        lhsT = rightT[b][:, jc * 128:(jc + 1) * 128]
                        stg = stag_pool.tile([128, 16 * 128], F32, tag="stag")
                        for il in range(4):
                            pb = ps_big.tile([128, 512], F32, tag="big")
                            nc.tensor.matmul(pb, lhsT, mps[il],
                                             start=True, stop=True)
                            dst = stg[:, il * 512:(il + 1) * 512]
                            if il % 2 == 0:
                                nc.vector.tensor_copy(out=dst, in_=pb)
                            else:
                                nc.scalar.copy(out=dst, in_=pb)
                        dst_ap = out[b, jc, sg, :, :, :]
                        src_ap = stg[:, :].rearrange("j (i p) -> j i p", p=128)
                        deng = nc.sync if jc % 2 == 0 else nc.gpsimd
                        deng.dma_start(out=dst_ap, in_=src_ap)

    nc.compile()
    names = ["node_full", "node_shard", "mask_col_full", "mask_col_shard",
             "mask_row_full", "mask_row_shard", "w_left_e", "w_right_e",
             "b_left_col", "b_right_col", "w2"]
    return nc, names


def _prepare_in_maps(node, mask, ln_gamma, ln_beta, W_left, b_left, W_right,
                     b_right, W_out, b_out):
    f = np.float32
    node = np.ascontiguousarray(np.asarray(node, dtype=f))        # [B, L, D]
    mask_f = np.asarray(mask).astype(f)                           # [B, L]
    gamma = np.asarray(ln_gamma, dtype=f)
    beta = np.asarray(ln_beta, dtype=f)
    W_l = np.asarray(W_left, dtype=f)
    W_r = np.asarray(W_right, dtype=f)
    b_l = np.asarray(b_left, dtype=f)
    b_r = np.asarray(b_right, dtype=f)
    W_o = np.asarray(W_out, dtype=f)
    b_o = np.asarray(b_out, dtype=f)

    s = 1.0 / np.sqrt(np.float32(DH))
    w_left_e = np.concatenate([gamma[:, None] * W_l, (beta @ W_l)[None, :]], 0)
    w_right_e = np.concatenate([gamma[:, None] * W_r, (beta @ W_r)[None, :]],
                               0) * s
    w2 = np.concatenate([np.repeat(W_o, 2, axis=0), b_o[None, :]], 0)

    node_flat = node.reshape(B * L, D)
    mask_col_full = np.ascontiguousarray(mask_f.reshape(-1, 128).T)  # [128, 8]
    mask_row_full = np.ascontiguousarray(mask_f)                     # [B, L]

    common = {
        "node_full": node_flat,
        "mask_col_full": mask_col_full,
        "mask_row_full": mask_row_full,
        "w_left_e": np.ascontiguousarray(w_left_e),
        "w_right_e": np.ascontiguousarray(w_right_e),
        "b_left_col": np.ascontiguousarray(b_l[:, None]),
        "b_right_col": np.ascontiguousarray(b_r[:, None] * s),
        "w2": np.ascontiguousarray(w2),
    }

    in_maps = []
    for c in range(NCORES):
        sl = slice(c * LSH, (c + 1) * LSH)
        shard = np.ascontiguousarray(node[:, sl, :].reshape(B * LSH, D))
        msk = mask_f[:, sl]                                       # [B, LSH]
        m = dict(common)
        m["node_shard"] = shard
        m["mask_col_shard"] = np.ascontiguousarray(msk.reshape(-1)[:, None])
        m["mask_row_shard"] = np.ascontiguousarray(msk.reshape(1, -1))
        in_maps.append(m)
    return in_maps


def kernel(**inputs):
    global _COMPILED
    if _COMPILED is None:
        _COMPILED = _build_program()
    nc, names = _COMPILED
    in_maps = _prepare_in_maps(**inputs)
    res = run_bass_kernel_spmd(nc, in_maps, core_ids=list(range(NCORES)))
    full = np.empty((B, L, L, PAIR), np.float32)
    for c in range(NCORES):
        dev = res.results[c]["out"]   # [b, jc, sg, j, i16, p]
        full[:, c * LSH:(c + 1) * LSH] = (
            dev.transpose(0, 2, 4, 1, 3, 5).reshape(B, LSH, L, PAIR))
    return full


if __name__ == "__main__":
    rng = np.random.default_rng(0)
    inputs = {
        "node": rng.standard_normal((B, L, D)).astype(np.float32),
        "mask": np.ones((B, L), dtype=bool),
        "ln_gamma": np.ones(D, np.float32),
        "ln_beta": np.zeros(D, np.float32),
        "W_left": (rng.standard_normal((D, DH)) / np.sqrt(D)).astype(np.float32),
        "b_left": np.zeros(DH, np.float32),
        "W_right": (rng.standard_normal((D, DH)) / np.sqrt(D)).astype(np.float32),
        "b_right": np.zeros(DH, np.float32),
        "W_out": (rng.standard_normal((H, PAIR)) / np.sqrt(H)).astype(np.float32),
        "b_out": np.zeros(PAIR, np.float32),
    }
    got = kernel(**inputs)
    print("kernel output:", got.shape, got.dtype)
